# revision 1
# baseline (speedup 1.0000x reference)
"""NeRF MLP kernel for Trainium2 (Bass/Tile), 8-core data-parallel over rays.

Layout: features on SBUF partitions, rays on the free dim; chunk = one
sample index s for all 512 local rays.

v2 design (fp8 DoubleRow):
- Hidden-layer matmuls run in fp8e4 DoubleRow: weights are host-scaled by
  512 (into e4m3 normal range), activations stored as 32*h in fp8, so each
  256-contraction layer half is ONE PE matmul.  PSUM holds 16384*preact.
- Biases are pre-accumulated INTO PSUM: the k=31 encoding matmuls carry
  [w;b]*16384 rows directly; L1/L2 biases ride four K=1 "bias matmuls"
  packed into one PE slot via tile_position row groups.  Post-matmul ops
  then become a single (max 0)*2^-9 tensor_scalar on any engine.
  L3/L5/L6/L7f halves instead use ScalarE activation (scale+bias+relu in
  one op) to balance engine load.
- Angle path: u2(s) = s*DStep + U0 per chunk (z_{s+1}-z_s = DELTA exactly,
  offsets cancel), so no per-chunk broadcast DMA; magic-add range
  reduction; one ScalarE Sin -> bf16 encoding.
- view-dir color term (w8v @ v3) is s-independent: hoisted to the
  pre-phase, added during phase-2 compositing.
- Compositing: w_s = exp(-cumsum) difference via triangular fp32 matmuls
  (exp(0)=1 exactly -> fully-transparent rays give exactly zero output).
- Density/color rows staged from PSUM by one engine copy each + SBUF->SBUF
  row-scatter DMA (DMA cannot read PSUM).
"""

import math
from contextlib import ExitStack

import numpy as np

import concourse.bass as bass
import concourse.mybir as mybir
import concourse.tile as tile
from concourse import bacc

F32 = mybir.dt.float32
BF16 = mybir.dt.bfloat16
FP8 = mybir.dt.float8e4
AF = mybir.ActivationFunctionType
OP = mybir.AluOpType
DR = mybir.MatmulPerfMode.DoubleRow

S = 64          # samples per ray
B_FULL = 4096   # total rays
N_CORES = 8
BL = B_FULL // N_CORES  # rays per core = 512
H = 256
NEAR, FAR = 2.0, 6.0
DELTA = (FAR - NEAR) / S
L_ENC = 5
ENC = 3 * L_ENC * 2  # 30
PI = math.pi
TWO_PI = 2.0 * math.pi
MAGIC = 12582912.0  # 1.5 * 2**23, fp32 round-to-nearest trick

WS = 512.0      # fp8 weight scale
AS = 32.0       # fp8 activation scale (stored act = 32*h)
PS = WS * AS    # psum scale = 16384
INV_PS_AS = AS / PS  # 2^-9: psum -> stored-act scale


def host_constants():
    c = {}
    freqs = (2.0 ** (np.arange(L_ENC, dtype=np.float64) - 2)) * math.pi  # [L]
    fturn = np.zeros((ENC, 1), dtype=np.float32)
    phase = np.zeros((ENC, 1), dtype=np.float32)
    for cc in range(3):
        for ll in range(L_ENC):
            for tt in range(2):
                j = cc * (L_ENC * 2) + ll * 2 + tt
                fturn[j, 0] = freqs[ll] / TWO_PI
                phase[j, 0] = 0.0 if tt == 0 else 0.25  # pi/2 in turns
    c["fturn30"] = fturn
    c["phase30"] = phase

    c["cap1e10"] = np.full((1, BL), 1.0e10, dtype=np.float32)
    c["svec64"] = (NEAR + np.arange(S, dtype=np.float32)[:, None] * DELTA).astype(
        np.float32
    )
    c["ltri"] = np.triu(np.ones((S, S), dtype=np.float32))  # lhsT[t,s]=1 for t<=s
    c["ltri2"] = (np.triu(np.ones((S, S))) + np.eye(S)).astype(np.float32)
    c["ones31"] = np.ones((3, 1), dtype=np.float32)
    c["ones641"] = np.ones((S, 1), dtype=np.float32)
    return c


def host_weights(inp):
    w = {}

    def kstack(m):  # [256, M] -> [128, 2, M]
        return np.ascontiguousarray(m.reshape(2, 128, m.shape[1]).transpose(1, 0, 2))

    # k=31 encoding matmuls carry weight+bias, scaled by PS (=16384).
    # 4 row-blocks of 32 (30 weights + bias row + zero pad), one per chunk
    # in a group of 4 -- the rhs enc4 tile holds 4 chunks' encodings at
    # partitions {0,32,64,96} and its row-30 pads evaluate to sin(pi/2)=1.
    def blk4(wmat, bvec):
        # block 2p+m (partition rows 32*(2p+m)..) = half m of the weights;
        # p = chunk parity within a 2-chunk group.  Lets the two halves'
        # k=31 matmuls use different PE row groups and pack concurrently.
        t = np.zeros((128, 128), dtype=np.float32)
        for p in range(2):
            for m in range(2):
                j = 2 * p + m
                t[32 * j : 32 * j + ENC] = wmat[:, m * 128 : (m + 1) * 128] * PS
                t[32 * j + ENC] = bvec[m * 128 : (m + 1) * 128] * PS
        return t

    w["w0x"] = blk4(inp["w0"], inp["b0"])                 # [128,128]
    w["w4ex"] = blk4(inp["w4"][H:H + ENC], inp["b4"])     # [128,128]

    # fp8 DoubleRow weights, scaled by WS (=512)
    for i in (1, 2, 3, 5, 6):
        w[f"wq{i}"] = kstack(inp[f"w{i}"]) * WS          # [128, 2, 256]
    w["w4h"] = kstack(inp["w4"][0:H]) * WS               # [128, 2, 256]
    w["w7f"] = kstack(inp["w7"][:, 1:129]) * WS          # [128, 2, 128]
    w7dp = np.zeros((256, 16), dtype=np.float32)
    w7dp[:, 0] = inp["w7"][:, 0] * WS
    w["w7d"] = kstack(w7dp)                              # [128, 2, 16] (padded)

    # L1/L2 bias rows for the packed K=1 bias matmuls (PS-scaled)
    bq = np.zeros((128, 128), dtype=np.float32)
    bq[0, :] = inp["b1"][0:128] * PS
    bq[32, :] = inp["b1"][128:256] * PS
    bq[64, :] = inp["b2"][0:128] * PS
    bq[96, :] = inp["b2"][128:256] * PS
    w["biasq"] = bq
    bq2 = np.zeros((128, 128), dtype=np.float32)
    bq2[0, :] = inp["b3"][0:128] * PS
    bq2[32, :] = inp["b3"][128:256] * PS
    w["biasq2"] = bq2

    # ScalarE-native biases (32*b), per-partition vectors
    for i in (5, 6):
        w[f"b{i}s32"] = np.ascontiguousarray(
            inp[f"b{i}"].reshape(2, 128).T) * AS          # [128, 2]
    w["b7f32"] = np.ascontiguousarray(inp["b7"][1:129, None]) * AS  # [128, 1]

    w["w8f3"] = inp["w8"][0:128] / AS                    # [128, 3]
    w["w8v3"] = inp["w8"][128:131]                       # [3, 3]
    w["b7d64"] = np.full((S, 1), inp["b7"][0], dtype=np.float32)
    for c in range(3):
        w[f"b8c64_{c}"] = np.full((S, 1), inp["b8"][c], dtype=np.float32)
    return w


def input_specs():
    specs = {
        "xT": (6, BL),
        "off": (S, BL),
        "w0x": (128, 128),
        "w4ex": (128, 128),
        "w4h": (128, 2, 256),
        "w7f": (128, 2, 128),
        "w7d": (128, 2, 16),
        "biasq": (128, 128),
        "biasq2": (128, 128),
        "b7f32": (128, 1),
        "w8f3": (128, 3),
        "w8v3": (3, 3),
        "b7d64": (S, 1),
        "b8c64_0": (S, 1),
        "b8c64_1": (S, 1),
        "b8c64_2": (S, 1),
        "cap1e10": (1, BL),
        "fturn30": (ENC, 1),
        "phase30": (ENC, 1),
        "svec64": (S, 1),
        "ltri": (S, S),
        "ltri2": (S, S),
        "ones31": (3, 1),
        "ones641": (S, 1),
    }
    for i in (1, 2, 3, 5, 6):
        specs[f"wq{i}"] = (128, 2, 256)
    for i in (5, 6):
        specs[f"b{i}s32"] = (128, 2)
    return specs


CONST_NAMES = tuple(n for n in input_specs() if n not in ("xT", "off"))

# name -> device dtype for PE-feeding tiles
FP8_CONSTS = ("wq1", "wq2", "wq3", "w4h", "wq5", "wq6", "w7f", "w7d")
BF16_CONSTS = ("w0x", "w4ex", "biasq", "biasq2", "w8f3", "w8v3")


def bcast_rows(ap, reps, cols):
    rows = ap.shape[0]
    return bass.AP(
        tensor=ap.tensor,
        offset=ap.offset,
        ap=[[ap.ap[0][0], rows], [0, reps], [1, cols]],
    )


def build_nerf(tc, ctx, out_ap, a, repeat=1, taps=None):
    nc = tc.nc
    B = BL

    consts = ctx.enter_context(tc.tile_pool(name="consts", bufs=1))
    pre = ctx.enter_context(tc.tile_pool(name="pre", bufs=1))
    work = ctx.enter_context(tc.tile_pool(name="work", bufs=3))
    psum = ctx.enter_context(tc.tile_pool(name="psum", bufs=1, space="PSUM"))

    # ---- load constants / weights into SBUF ----
    sb = {}
    for name in CONST_NAMES:
        t = consts.tile(list(a[name].shape), F32, name=name, tag=name)
        nc.sync.dma_start(out=t, in_=a[name])
        sb[name] = t
    sr = {}
    for name in FP8_CONSTS:
        t = consts.tile(list(a[name].shape), FP8, name=name + "_q", tag=name + "_q")
        nc.vector.tensor_copy(t, sb[name])
        sr[name] = t
    for name in BF16_CONSTS:
        t = consts.tile(list(a[name].shape), BF16, name=name + "_b", tag=name + "_b")
        nc.vector.tensor_copy(t, sb[name])
        sr[name] = t
    ones4 = consts.tile([128, B], BF16, name="ones4", tag="ones4")
    nc.vector.memset(ones4, 1.0)

    dt3 = pre.tile([3, B], F32, name="dt3", tag="dt3")
    nc.sync.dma_start(out=dt3, in_=a["xT"][3:6])
    off = pre.tile([S, B], F32, name="off", tag="off")
    nc.sync.dma_start(out=off, in_=a["off"])

    # per-ray encoding constants: angle/2pi = DF*z + AO  (30 rows)
    D30 = pre.tile([ENC, B], F32, name="D30", tag="D30")
    nc.sync.dma_start(out=D30, in_=bcast_rows(a["xT"][3:6], 2 * L_ENC, B))
    O30 = pre.tile([ENC, B], F32, name="O30", tag="O30")
    nc.sync.dma_start(out=O30, in_=bcast_rows(a["xT"][0:3], 2 * L_ENC, B))
    DF = pre.tile([ENC, B], F32, name="DF", tag="DF")
    nc.vector.tensor_scalar(out=DF, in0=D30, scalar1=sb["fturn30"],
                            scalar2=None, op0=OP.mult)
    AO = pre.tile([ENC, B], F32, name="AO", tag="AO")
    nc.vector.tensor_scalar(out=AO, in0=O30, scalar1=sb["fturn30"],
                            scalar2=sb["phase30"], op0=OP.mult, op1=OP.add)

    # Z[s, b] = NEAR + (s + off) * DELTA
    Z = pre.tile([S, B], F32, name="Z", tag="Z")
    nc.vector.tensor_scalar(out=Z, in0=off, scalar1=DELTA, scalar2=sb["svec64"],
                            op0=OP.mult, op1=OP.add)

    # |d| and 1/|d|
    sq3 = pre.tile([3, B], F32, name="sq3", tag="sq3")
    nc.vector.tensor_mul(sq3, dt3, dt3)
    ps_nd = psum.tile([128, 512], F32, name="ps", tag="ps", bufs=2)[0:1, :B]
    nc.tensor.matmul(ps_nd, sb["ones31"], sq3, start=True, stop=True)
    nd = pre.tile([1, B], F32, name="nd", tag="nd")
    nc.scalar.activation(out=nd, in_=ps_nd, func=AF.Sqrt)
    inv_nd = pre.tile([1, B], F32, name="inv_nd", tag="inv_nd")
    nc.vector.reciprocal(out=inv_nd, in_=nd)

    # view_dir = d / |d| (bf16), and its color contribution vc3 = w8v.T @ v3
    inv3 = pre.tile([3, B], F32, name="inv3", tag="inv3")
    nc.gpsimd.partition_broadcast(inv3, inv_nd)
    v3 = pre.tile([3, B], BF16, name="v3", tag="v3")
    nc.vector.tensor_mul(v3, dt3, inv3)
    ps_vc = psum.tile([128, 512], F32, name="ps", tag="ps", bufs=2)[0:3, :B]
    nc.tensor.matmul(ps_vc, sr["w8v3"], v3, start=True, stop=True)
    vc3 = pre.tile([3, B], F32, name="vc3", tag="vc3")
    nc.scalar.activation(out=vc3, in_=ps_vc, func=AF.Copy)
    vcb = []
    for c in range(3):
        t = pre.tile([S, B], F32, name=f"vcb{c}", tag=f"vcb{c}")
        nc.sync.dma_start(out=t, in_=bcast_rows(vc3[c:c + 1], S, B))
        vcb.append(t)

    # dists
    nd64 = pre.tile([S, B], F32, name="nd64", tag="nd64")
    nc.gpsimd.partition_broadcast(nd64, nd)
    ZN = pre.tile([S, B], F32, name="ZN", tag="ZN")
    nc.vector.tensor_mul(ZN, Z, nd64)
    ZNs = pre.tile([S, B], F32, name="ZNs", tag="ZNs")
    nc.sync.dma_start(out=ZNs[0 : S - 1], in_=ZN[1:S])
    nc.sync.dma_start(out=ZNs[S - 1 : S], in_=a["cap1e10"])
    dists = pre.tile([S, B], F32, name="dists", tag="dists")
    nc.vector.tensor_sub(dists, ZNs, ZN)

    # angle-path linear recurrence constants: u2(s) = s*DStep + U0.
    # 4-block layout [128,B]: block j (partitions 32j..32j+31) carries chunk
    # s0+j of a group: rows 0..29 = U0 + j*DStep / DStep; row 30,31 pads are
    # (0.25, 0) so the fractional angle is exactly 0.25 turns -> sin = 1.0,
    # which feeds the k=31 matmuls' bias rows.
    z0b = pre.tile([ENC, B], F32, name="z0b", tag="z0b")
    nc.gpsimd.partition_broadcast(z0b, Z[0:1])
    U0 = pre.tile([ENC, B], F32, name="U0", tag="U0")
    nc.vector.tensor_mul(U0, DF, z0b)
    nc.vector.tensor_add(U0, U0, AO)
    DStep = pre.tile([ENC, B], F32, name="DStep", tag="DStep")
    nc.vector.tensor_scalar(out=DStep, in0=DF, scalar1=float(DELTA),
                            scalar2=None, op0=OP.mult)
    U04 = pre.tile([128, B], F32, name="U04", tag="U04")
    nc.vector.memset(U04, 0.25)
    DStep4 = pre.tile([128, B], F32, name="DStep4", tag="DStep4")
    nc.vector.memset(DStep4, 0.0)
    u0b = pre.tile([ENC, B], F32, name="u0b", tag="u0b")
    nc.vector.tensor_add(u0b, U0, DStep)
    for j in range(4):
        # affine_then_add mis-writes at nonzero base partitions on HW,
        # so compute at base 0 and tensor_copy into the blocks.
        nc.vector.tensor_copy(U04[32 * j : 32 * j + ENC], U0 if j < 2 else u0b)
        nc.vector.tensor_copy(DStep4[32 * j : 32 * j + ENC], DStep)

    # phase-2 accumulators
    D64 = pre.tile([S, B], F32, name="D64", tag="D64")
    TH2 = pre.tile([S, 3, B], F32, name="TH2", tag="TH2")

    # ---- chunk loop ----
    # Groups of 4 samples share one angle/sin op set (partition-packed).
    # Within a group, chunks are processed as two INTERLEAVED pairs: each
    # stage (matmuls + psum evacuation) is emitted for chunk a then chunk b,
    # so engine FIFOs overlap one chunk's evacs with the other's matmuls --
    # without this the psum pool serializes the whole kernel into one
    # MM->evac chain.  Bias matmuls are deferred to their layer's stage to
    # keep at most ~2 psum pairs live per stream.
    assert S % 4 == 0

    def mk_state(s, j, enc4, eng):
        return {"s": s, "rb": 32 * j, "enc": enc4, "h": {}, "p": {}, "eng": eng}

    def evac_merged(eng, out, psum_ap):
        if eng == "act":
            nc.scalar.activation(out=out, in_=psum_ap, func=AF.Relu,
                                 scale=INV_PS_AS)
        else:
            nc.vector.tensor_scalar(out=out, in0=psum_ap, scalar1=0.0,
                                    scalar2=INV_PS_AS, op0=OP.max, op1=OP.mult)

    def st_l0(st):
        rb = st["rb"]
        p = psum.tile([128, 2, 512], F32, name="pp", tag="pp", bufs=3)
        for m in range(2):
            rbm = rb + 32 * m
            nc.tensor.matmul(p[:, m, :B],
                             sr["w0x"][rbm : rbm + ENC + 1, :],
                             st["enc"][rbm : rbm + ENC + 1],
                             start=True, stop=True, tile_position=(rbm, 0))
        h0 = work.tile([128, 2, B], FP8, name="h0", tag="h", bufs=10)
        evac_merged(st["eng"]["l0"], h0, p[:, :, :B])
        st["h"][0] = h0

    def mk_dr(li, bias_tile, bias_rows):
        def st_dr(st):
            p = psum.tile([128, 2, 512], F32, name="pp", tag="pp", bufs=3)
            for m in range(2):
                nc.tensor.matmul(p[:, m, :B], bias_tile[bias_rows[m] : bias_rows[m] + 1, :],
                                 ones4[bias_rows[m] : bias_rows[m] + 1, :],
                                 start=True, stop=False,
                                 tile_position=(bias_rows[m], 0))
            for m in range(2):
                nc.tensor.matmul(p[:, m, :B],
                                 sr[f"wq{li}"][:, :, m * 128 : (m + 1) * 128],
                                 st["h"][li - 1], start=False, stop=True,
                                 perf_mode=DR)
            h = work.tile([128, 2, B], FP8, name=f"h{li}", tag="h", bufs=10)
            evac_merged(st["eng"][f"l{li}"], h, p[:, :, :B])
            st["h"][li] = h
        return st_dr

    def st_l4(st):
        rb = st["rb"]
        p = psum.tile([128, 2, 512], F32, name="pp", tag="pp", bufs=3)
        h4 = work.tile([128, 2, B], FP8, name="h4", tag="h", bufs=10)
        for m in range(2):
            rbm = rb + 32 * m
            nc.tensor.matmul(p[:, m, :B],
                             sr["w4ex"][rbm : rbm + ENC + 1, :],
                             st["enc"][rbm : rbm + ENC + 1],
                             start=True, stop=False, tile_position=(rbm, 0))
        for m in range(2):
            nc.tensor.matmul(p[:, m, :B],
                             sr["w4h"][:, :, m * 128 : (m + 1) * 128],
                             st["h"][3], start=False, stop=True, perf_mode=DR)
        evac_merged(st["eng"]["l4"], h4, p[:, :, :B])
        st["h"][4] = h4

    def mk_split(li):
        def st_split(st):
            p = psum.tile([128, 2, 512], F32, name="pp", tag="pp", bufs=3)
            h = work.tile([128, 2, B], FP8, name=f"h{li}", tag="h", bufs=10)
            for m in range(2):
                nc.tensor.matmul(p[:, m, :B],
                                 sr[f"wq{li}"][:, :, m * 128 : (m + 1) * 128],
                                 st["h"][li - 1], start=True, stop=True,
                                 perf_mode=DR)
                nc.scalar.activation(out=h[:, m, :], in_=p[:, m, :B],
                                     func=AF.Relu, scale=INV_PS_AS,
                                     bias=sb[f"b{li}s32"][:, m : m + 1])
            st["h"][li] = h
        return st_split

    def st_l7f(st):
        p7f = psum.tile([128, 512], F32, name="ps", tag="ps", bufs=2)[:, :B]
        nc.tensor.matmul(p7f, sr["w7f"], st["h"][6], start=True, stop=True,
                         perf_mode=DR)
        F1 = work.tile([128, B], BF16, name="F1", tag="F1", bufs=4)
        nc.scalar.activation(out=F1, in_=p7f, func=AF.Relu,
                             scale=INV_PS_AS, bias=sb["b7f32"])
        st["F1"] = F1

    def st_tail(st):
        s = st["s"]
        pd8 = psum.tile([128, 2, 512], F32, name="pp", tag="pp", bufs=3)
        nc.tensor.matmul(pd8[0:16, 0, :B], sr["w7d"], st["h"][6],
                         start=True, stop=True, perf_mode=DR)
        nc.tensor.matmul(pd8[0:3, 1, :B], sr["w8f3"], st["F1"],
                         start=True, stop=True)
        dth = work.tile([3, 2, B], F32, name="dth", tag="dth", bufs=4)
        if st["eng"]["dth"] == "act":
            nc.scalar.activation(out=dth, in_=pd8[0:3, :, :B], func=AF.Copy)
        else:
            nc.vector.tensor_copy(dth, pd8[0:3, :, :B])
        nc.sync.dma_start(out=D64[s : s + 1], in_=dth[0:1, 0, :])
        nc.sync.dma_start(out=TH2[s : s + 1, :, :], in_=dth[0:3, 1, :])

    stages = [
        st_l0,
        mk_dr(1, sr["biasq"], (0, 32)),
        mk_dr(2, sr["biasq"], (64, 96)),
        mk_dr(3, sr["biasq2"], (0, 32)),
        st_l4,
        mk_split(5),
        mk_split(6),
        st_l7f,
        st_tail,
    ]

    # per-stream engine maps, phase-shifted so one stream loads the
    # ScalarE while the other loads the DVE
    ENG_A = {"l0": "act", "l1": "vec", "l2": "vec", "l3": "vec",
             "l4": "act", "dth": "vec"}
    ENG_B = {"l0": "vec", "l1": "act", "l2": "act", "l3": "vec",
             "l4": "vec", "dth": "vec"}

    for g in range(S // 2 * repeat):
        s0 = (g * 2) % S
        u24 = work.tile([128, B], F32, name="u24", tag="u24")
        nc.vector.affine_then_add(out=u24, in0=DStep4, in1=U04,
                                  scale=float(s0), bias=0.0)
        kk4 = work.tile([128, B], F32, name="kk4", tag="kk4")
        nc.vector.tensor_scalar(out=kk4, in0=u24, scalar1=MAGIC, scalar2=MAGIC,
                                op0=OP.add, op1=OP.subtract)
        f4 = work.tile([128, B], F32, name="f4", tag="f4")
        nc.vector.tensor_sub(f4, u24, kk4)
        enc4 = work.tile([128, B], BF16, name="enc4", tag="enc4")
        nc.scalar.activation(out=enc4, in_=f4, func=AF.Sin, scale=TWO_PI)
        if taps is not None and g == 0:
            taps["enc4"] = enc4
            taps["u24"] = u24
            taps["f4"] = f4

        sa = mk_state(s0, 0, enc4, ENG_A)
        sb_ = mk_state(s0 + 1, 2, enc4, ENG_B)
        for stage in stages:
            stage(sa)
            stage(sb_)

    if taps is not None:
        taps["U04"] = U04
        taps["DStep4"] = DStep4
        taps["D64"] = D64
        taps["TH2"] = TH2
        for tname, t in taps.items():
            o = a["tap_" + tname]
            nc.sync.dma_start(out=o, in_=t)

    # ---- phase 2: compositing in [64, B] layout ----
    SG = pre.tile([S, B], F32, name="SG", tag="SG")
    nc.scalar.activation(out=SG, in_=D64, func=AF.Relu, bias=sb["b7d64"],
                         scale=1.0 / PS)
    M64 = pre.tile([S, B], F32, name="M64", tag="M64")
    nc.vector.tensor_mul(M64, SG, dists)

    # weight_s = exp(-cumsum_{t<s}) - exp(-cumsum_{t<=s})
    mcum = psum.tile([128, 512], F32, name="ps", tag="ps", bufs=2)[:S, :B]
    nc.tensor.matmul(mcum, sb["ltri"], M64, start=True, stop=True)
    vcum = psum.tile([128, 512], F32, name="ps", tag="ps", bufs=2)[:S, :B]
    nc.tensor.matmul(vcum, sb["ltri2"], M64, start=True, stop=True)
    T64 = pre.tile([S, B], F32, name="T64", tag="T64")
    nc.scalar.activation(out=T64, in_=mcum, func=AF.Exp, scale=-1.0)
    T64b = pre.tile([S, B], F32, name="T64b", tag="T64b")
    nc.scalar.activation(out=T64b, in_=vcum, func=AF.Exp, scale=-1.0)
    w64 = pre.tile([S, B], F32, name="w64", tag="w64")
    nc.vector.tensor_sub(w64, T64, T64b)

    # out_c = sum_s w_s * sigmoid(th_c + vc3_c + b8_c)
    for c in range(3):
        tc_ = pre.tile([S, B], F32, name=f"tc{c}", tag=f"tc{c}")
        nc.vector.tensor_add(tc_, TH2[:, c, :], vcb[c])
        St = pre.tile([S, B], F32, name=f"St{c}", tag=f"St{c}")
        nc.scalar.activation(out=St, in_=tc_, func=AF.Sigmoid,
                             bias=sb[f"b8c64_{c}"])
        P = pre.tile([S, B], F32, name=f"P{c}", tag=f"P{c}")
        nc.vector.tensor_mul(P, w64, St)
        pc = psum.tile([128, 512], F32, name="ps", tag="ps", bufs=2)[0:1, :B]
        nc.tensor.matmul(pc, sb["ones641"], P, start=True, stop=True)
        oc = pre.tile([1, B], F32, name=f"oc{c}", tag=f"oc{c}")
        nc.scalar.activation(out=oc, in_=pc, func=AF.Copy)
        nc.sync.dma_start(out=out_ap.rearrange("b c -> c b")[c : c + 1], in_=oc)


TAP_SPECS = {
    "enc4": ([128, BL], BF16), "u24": ([128, BL], F32), "f4": ([128, BL], F32),
    "U04": ([128, BL], F32), "DStep4": ([128, BL], F32),
    "D64": ([S, BL], F32), "TH2": ([S, 3, BL], F32),
}


def build_nc(repeat=1, debug_taps=False):
    nc = bacc.Bacc("TRN2", target_bir_lowering=False, debug=False)
    specs = input_specs()
    aps = {
        name: nc.dram_tensor(name, list(shape), F32, kind="ExternalInput").ap()
        for name, shape in specs.items()
    }
    taps = None
    if debug_taps:
        taps = {}
        for tname, (shape, dt) in TAP_SPECS.items():
            aps["tap_" + tname] = nc.dram_tensor(
                "tap_" + tname, shape, dt, kind="ExternalOutput").ap()
    out = nc.dram_tensor("out", [BL, 3], F32, kind="ExternalOutput").ap()
    with tile.TileContext(nc) as tc, ExitStack() as ctx:
        build_nerf(tc, ctx, out, aps, repeat=repeat, taps=taps)
    nc.compile()
    return nc


def make_in_maps(inputs):
    consts = host_constants()
    wts = host_weights(inputs)
    shared = {**consts, **wts}
    shared = {k: np.ascontiguousarray(v, dtype=np.float32) for k, v in shared.items()}
    in_maps = []
    for core in range(N_CORES):
        sl = slice(core * BL, (core + 1) * BL)
        m = dict(shared)
        m["xT"] = np.ascontiguousarray(np.asarray(inputs["x"])[sl].T, dtype=np.float32)
        m["off"] = np.ascontiguousarray(
            np.asarray(inputs["offsets"])[:, sl], dtype=np.float32
        )
        in_maps.append(m)
    return in_maps


def kernel(**inputs):
    from concourse.bass_utils import run_bass_kernel_spmd

    nc = build_nc()
    in_maps = make_in_maps(inputs)
    res = run_bass_kernel_spmd(nc, in_maps, core_ids=list(range(N_CORES)))
    out = np.concatenate([r["out"] for r in res.results], axis=0)
    return out.astype(np.float32)

